# revision 52
# baseline (speedup 1.0000x reference)
"""MultiHeadDistanceKNN Trainium2 kernel.

kernel(x, W) -> adj : x [2,2048,512] f32, W [4,512,128] f32 -> adj [2,2048,2048] f32.

8 cores = 4 heads x 2 batches; core i handles (h=i//2, b=i%2) and computes
  C[n,m] = exp(-d2[n,m]/(2*mu^2)) * 1[d2[n,m] <= max(T_n, T_m)]
where d2 = |z_n - z_m|^2 for z = x_b @ W_h (exactly symmetric), T_n = K-th
smallest of row n (K=307), mu = mean distance. Host: adj[b] = mean_h C.

Pipeline: prep streams x chunks (sync DMA queue), transposes them on the PE
and fires each z j-tile as soon as its 4 chunks land. d2 built on the PE
(bf16-split 3-term matmul + K=4 aug for the norms), ACT relu-copies psum ->
SBUF D2. Bracket counts are fused into the build (DVE is_le+accum at hi, ACT
Sign+accum at lo -> exact clo); the build runs in two 8-chunk groups so probe
round 0 for group A hides under group B's matmuls. 4 more secant rounds
(DVE/ACT split with per-engine scratch), then a top-8-below-hi finisher:
wscr = (d2<=hi)*d2 in one DVE STT, MAX8, T = w8[chi - K] (no recount - chi
is tracked exactly through the rounds).
Final: sim = ACT exp (bf16), mask = STT(max(TROWB,T_n) >= d2) bf16, C = bf16
tensor_mul (DVE 2x mode; GpSimd deliberately idle - its big ops stall DVE via
the shared SBUF ports), bf16 DMA out; host upcasts and head-means. The
identity for PE transposes ships as a kernel input (GPS setup ops would
delay the first transposes ~8us).
"""
import numpy as np

import concourse.bass as bass
import concourse.mybir as mybir
from concourse import bacc
from concourse.tile import TileContext

F32 = mybir.dt.float32
BF16 = mybir.dt.bfloat16
U8 = mybir.dt.uint8
Alu = mybir.AluOpType
Act = mybir.ActivationFunctionType
X_AX = mybir.AxisListType.X
XY_AX = mybir.AxisListType.XY

N = 2048
D = 512
NCH = 16
NJT = 4
K = 307

N_ROUNDS = 4          # probe rounds total (round 0 split across build groups)
AIM = 3.5
ZLO = -1.45
ZHI = -0.70
DVE_CNT = 7           # count chunks on DVE per full round (rest on ACT)
ACT_WSCR = 0          # finisher wscr chunks via ACT sigmoid + GPS mult
BIGSIG = 4.0e4


def build_nc():
    nc = bacc.Bacc("TRN2", target_bir_lowering=False)
    xb = nc.dram_tensor("xb", [N, D], F32, kind="ExternalInput")
    wh = nc.dram_tensor("wh", [D, 128], F32, kind="ExternalInput")
    idm = nc.dram_tensor("idm", [128, 128], F32, kind="ExternalInput")
    outp = nc.dram_tensor("outp", [N, N], BF16, kind="ExternalOutput")

    with TileContext(nc) as tc:
        with tc.tile_pool(name="base", bufs=1) as base, \
             tc.tile_pool(name="st", bufs=1) as st:
            D2 = base.tile([128, NCH * N], F32)
            ident = base.tile([128, 128], F32)
            nc.sync.dma_start(ident[:], idm[:, :])
            ones_col = base.tile([128, 1], F32)
            nc.vector.memset(ones_col[:], 1.0)
            ones_row = base.tile([1, 128], F32)
            nc.vector.memset(ones_row[:], 1.0)
            id1 = base.tile([1, 1], F32)
            nc.vector.memset(id1[:], 1.0)
            c256 = base.tile([128, 1], F32)
            nc.vector.memset(c256[:], 256.0)

            def stt16(name, w=NCH, dt=F32):
                return st.tile([128, w], dt, tag=name, name=name)
            sqcol = stt16("sqcol"); zdots = stt16("zdots")
            mu = stt16("mu"); sig = stt16("sig")
            width0 = stt16("width0")
            # paired state, ping-pong: [:, 0:16] = value, [:, 16:32] = count
            lc = [stt16("lcA", 32), stt16("lcB", 32)]     # (lo, clo)
            hc = [stt16("hcA", 32), stt16("hcB", 32)]     # (hi, chi)
            chieff = [stt16("ceA"), stt16("ceB")]
            tcnt = stt16("tcnt", 32)                       # (t, cnt)
            negt = stt16("negt"); sacc = stt16("sacc")
            neglo = stt16("neglo"); lacc = stt16("lacc")
            musum = stt16("musum")
            Tfin = stt16("Tfin")
            bighi = stt16("bighi")
            tmp1 = stt16("tmp1"); tmp2 = stt16("tmp2"); tmp3 = stt16("tmp3")
            tmp4 = stt16("tmp4")
            mge = stt16("mge", NCH, U8)
            mbh = stt16("mbh", NCH, U8)
            mok = stt16("mok", NCH, U8)
            s_vec = st.tile([128, 1], F32, tag="s_vec", name="s_vec")
            s1b = st.tile([128, 1], F32, tag="s1b", name="s1b")
            neginvb = st.tile([128, 1], F32, tag="neginvb", name="neginvb")
            sc1 = st.tile([1, 1], F32, tag="sc1", name="sc1")
            sc2 = st.tile([1, 1], F32, tag="sc2", name="sc2")
            sc3 = st.tile([1, 1], F32, tag="sc3", name="sc3")
            sc4 = st.tile([1, 1], F32, tag="sc4", name="sc4")
            w8 = st.tile([128, NCH * 8], F32, tag="w8", name="w8")
            iota8f = st.tile([128, NCH * 8], F32, tag="iota8f", name="iota8f")
            ohsel = st.tile([128, NCH * 8], F32, tag="ohsel", name="ohsel")

            # ============== prep: stream x, transpose, z, norms ==============
            with tc.tile_pool(name="mid", bufs=1) as mid:
                sqrow = mid.tile([1, N], F32, tag="sqrow", name="sqrow")
                zpool = tc.tile_pool(name="zp", bufs=1)
                zp = zpool.__enter__()
                zT = zp.tile([128, N], F32, tag="zT", name="zT")
                zh = zp.tile([128, N], BF16, tag="zh", name="zh")
                zl = zp.tile([128, N], BF16, tag="zl", name="zl")
                vh = zp.tile([128, N], BF16, tag="vh", name="vh")
                vl = zp.tile([128, N], BF16, tag="vl", name="vl")

                with tc.tile_pool(name="prep", bufs=3) as prep, \
                     tc.tile_pool(name="prep1", bufs=1) as prep1, \
                     tc.tile_pool(name="pps", bufs=2, space="PSUM") as pps, \
                     tc.tile_pool(name="pps1", bufs=4, space="PSUM") as pps1:
                    w_sb = prep1.tile([128, D], F32)
                    xt = [prep1.tile([128, N], F32, tag=f"xt{dc}", name=f"xt{dc}")
                          for dc in range(4)]
                    for j in range(NJT):
                        js = slice(j * 512, (j + 1) * 512)
                        for cc in range(4):
                            c = 4 * j + cc
                            x_sb = prep.tile([128, D], F32, tag="x_sb",
                                             name="x_sb")
                            nc.sync.dma_start(x_sb[:],
                                              xb[c * 128:(c + 1) * 128, :])
                            for dc in range(4):
                                tr_ps = pps1.tile([128, 128], F32, tag="small",
                                                  name="tr")
                                nc.tensor.transpose(
                                    tr_ps[:], x_sb[:, dc * 128:(dc + 1) * 128],
                                    ident[:])
                                if dc % 2 == 0:
                                    nc.scalar.copy(
                                        xt[dc][:, c * 128:(c + 1) * 128],
                                        tr_ps[:])
                                else:
                                    nc.vector.tensor_copy(
                                        xt[dc][:, c * 128:(c + 1) * 128],
                                        tr_ps[:])
                        if j == 0:
                            for dc in range(4):
                                nc.sync.dma_start(
                                    w_sb[:, dc * 128:(dc + 1) * 128],
                                    wh[dc * 128:(dc + 1) * 128, :])
                        zt_ps = pps.tile([128, 512], F32, tag="big", name="zt")
                        for dc in range(4):
                            nc.tensor.matmul(
                                zt_ps[:], w_sb[:, dc * 128:(dc + 1) * 128],
                                xt[dc][:, js],
                                start=(dc == 0), stop=(dc == 3))
                        nc.vector.tensor_copy(zT[:, js], zt_ps[:])
                        # bf16 split per j-tile (DVE, overlaps PE)
                        nc.vector.tensor_copy(zh[:, js], zT[:, js])
                        nc.vector.tensor_sub(zl[:, js], zT[:, js], zh[:, js])
                        nc.vector.tensor_scalar(vh[:, js], zh[:, js], -2.0,
                                                scalar2=None, op0=Alu.mult)
                        nc.vector.tensor_scalar(vl[:, js], zl[:, js], -2.0,
                                                scalar2=None, op0=Alu.mult)
                        # squared norms per j-tile (ACT square into D2 scratch)
                        zT2j = D2[:, j * 512:(j + 1) * 512]
                        nc.scalar.activation(zT2j, zT[:, js], Act.Square)
                        sq_ps = pps1.tile([1, 512], F32, tag="small",
                                          name="sqps")
                        nc.tensor.matmul(sq_ps[:], ones_col[:], zT2j,
                                         start=True, stop=True)
                        nc.vector.tensor_copy(sqrow[0:1, js], sq_ps[:])

                # ---- norms + bracket init (emitted inside the build so
                # its serial math hides under chunk-0's matmuls) ----
                def emit_norms(pp):
                    for c in range(NCH):
                        tp = pp.tile([128, 1], F32, tag="small", name="sqcolp")
                        nc.tensor.transpose(tp[:],
                                            sqrow[0:1, c * 128:(c + 1) * 128],
                                            id1[:])
                        nc.vector.tensor_copy(sqcol[:, c:c + 1], tp[:])
                    nc.vector.tensor_reduce(s_vec[:], zT[:], axis=X_AX,
                                            op=Alu.add)
                    for c in range(NCH):
                        zd_ps = pp.tile([128, 1], F32, tag="small", name="zdps")
                        nc.tensor.matmul(zd_ps[:], zT[:, c * 128:(c + 1) * 128],
                                         s_vec[:], start=True, stop=True)
                        nc.vector.tensor_copy(zdots[:, c:c + 1], zd_ps[:])
                    nc.vector.tensor_reduce(sc1[:], sqrow[0:1, :], axis=X_AX,
                                            op=Alu.add)
                    s1_ps = pp.tile([128, 1], F32, tag="small", name="s1ps")
                    nc.tensor.matmul(s1_ps[:], ones_row[:], sc1[:],
                                     start=True, stop=True)
                    nc.vector.tensor_scalar(s1b[:], s1_ps[:], 1.0 / N,
                                            scalar2=None, op0=Alu.mult)
                    nc.vector.scalar_tensor_tensor(
                        out=mu[:], in0=zdots[:], scalar=-2.0 / N, in1=sqcol[:],
                        op0=Alu.mult, op1=Alu.add)
                    nc.vector.tensor_scalar(mu[:], mu[:], s1b[:], scalar2=None,
                                            op0=Alu.add)
                    # sig = sqrt(4*sqcol+256) linearized (keeps the norm
                    # chain DVE-only so ACT's in-order queue never head-blocks)
                    nc.vector.tensor_scalar(sig[:], sqcol[:], 0.07239054,
                                            scalar2=18.446953,
                                            op0=Alu.mult, op1=Alu.add)
                    nc.vector.scalar_tensor_tensor(
                        out=lc[0][:, 0:NCH], in0=sig[:], scalar=ZLO, in1=mu[:],
                        op0=Alu.mult, op1=Alu.add)
                    nc.vector.scalar_tensor_tensor(
                        out=hc[0][:, 0:NCH], in0=sig[:], scalar=ZHI, in1=mu[:],
                        op0=Alu.mult, op1=Alu.add)
                    nc.vector.tensor_sub(width0[:], hc[0][:, 0:NCH],
                                         lc[0][:, 0:NCH])
                    nc.vector.tensor_scalar(neglo[:], lc[0][:, 0:NCH], -1.0,
                                            scalar2=None, op0=Alu.mult)

                # ---- secant round helper (column-sliced) ----
                def emit_round(r, c0, c1, dve_k, cntD, cntA_scr,
                               defer_tail=False):
                    A, B = r % 2, (r + 1) % 2
                    sl = slice(c0, c1)
                    slc = slice(NCH + c0, NCH + c1)
                    loA, cloA = lc[A][:, sl], lc[A][:, slc]
                    hiA, chiA = hc[A][:, sl], hc[A][:, slc]
                    tA, cntsl = tcnt[:, sl], tcnt[:, slc]
                    # t = lo + clamp((K+AIM-clo)/(chieff-clo), .02, .98)*(hi-lo)
                    nc.vector.tensor_sub(tmp1[:, sl], chieff[A][:, sl], cloA)
                    nc.vector.tensor_scalar(tmp1[:, sl], tmp1[:, sl], 1.0,
                                            scalar2=None, op0=Alu.max)
                    nc.vector.reciprocal(tmp1[:, sl], tmp1[:, sl])
                    nc.vector.tensor_scalar(tmp2[:, sl], cloA, -1.0,
                                            scalar2=float(K) + AIM,
                                            op0=Alu.mult, op1=Alu.add)
                    nc.vector.tensor_mul(tmp1[:, sl], tmp1[:, sl], tmp2[:, sl])
                    nc.vector.tensor_scalar(tmp1[:, sl], tmp1[:, sl], 0.02,
                                            scalar2=0.98,
                                            op0=Alu.max, op1=Alu.min)
                    nc.vector.tensor_sub(tmp2[:, sl], hiA, loA)
                    nc.vector.tensor_mul(tmp1[:, sl], tmp1[:, sl], tmp2[:, sl])
                    nc.vector.tensor_add(tA, tmp1[:, sl], loA)
                    repair = r < 4
                    if repair:
                        nc.vector.tensor_scalar(mbh[:, sl], chiA, float(K),
                                                scalar2=None, op0=Alu.is_lt)
                        nc.vector.scalar_tensor_tensor(
                            out=tmp3[:, sl], in0=width0[:, sl],
                            scalar=float(2.0 ** r),
                            in1=hiA, op0=Alu.mult, op1=Alu.add)
                        nc.vector.select(tA, mbh[:, sl], tmp3[:, sl], tA)
                    nc.vector.tensor_scalar(negt[:, sl], tA, -1.0,
                                            scalar2=None, op0=Alu.mult)
                    # counts: first dve_k chunks on DVE, rest on ACT
                    act_cs = []
                    for c in range(c0, c1):
                        d2c = D2[:, c * N:(c + 1) * N]
                        if c - c0 < dve_k:
                            nc.vector.tensor_scalar(
                                cntD[:], d2c, tcnt[:, c:c + 1], scalar2=None,
                                op0=Alu.is_le, op1=Alu.add,
                                accum_out=tcnt[:, NCH + c:NCH + c + 1])
                        else:
                            nc.scalar.activation(cntA_scr[:], d2c, Act.Sign,
                                                 bias=negt[:, c:c + 1],
                                                 accum_out=sacc[:, c:c + 1])
                            act_cs.append(c)
                    def tail():
                        if act_cs:
                            a0, a1 = act_cs[0], act_cs[-1] + 1
                            nc.vector.tensor_scalar(
                                tcnt[:, NCH + a0:NCH + a1], sacc[:, a0:a1],
                                -0.5, scalar2=float(N) * 0.5,
                                op0=Alu.mult, op1=Alu.add)
                        # updates -> B set
                        nc.vector.tensor_scalar(mge[:, sl], cntsl, float(K),
                                                scalar2=None, op0=Alu.is_ge)
                        nc.vector.select(lc[B][:, sl], mge[:, sl], loA, tA)
                        nc.vector.select(lc[B][:, slc], mge[:, sl], cloA, cntsl)
                        nc.vector.select(hc[B][:, sl], mge[:, sl], tA, hiA)
                        nc.vector.select(hc[B][:, slc], mge[:, sl], cntsl, chiA)
                        if repair:
                            nc.vector.select(lc[B][:, sl], mbh[:, sl], hiA,
                                             lc[B][:, sl])
                            nc.vector.select(lc[B][:, slc], mbh[:, sl], chiA,
                                             lc[B][:, slc])
                            nc.vector.select(hc[B][:, sl], mbh[:, sl], tA,
                                             hc[B][:, sl])
                            nc.vector.select(hc[B][:, slc], mbh[:, sl], cntsl,
                                             hc[B][:, slc])
                        nc.vector.tensor_scalar(tmp4[:, sl], chieff[A][:, sl],
                                                0.5, scalar2=float(K) * 0.5,
                                                op0=Alu.mult, op1=Alu.add)
                        nc.vector.select(chieff[B][:, sl], mge[:, sl],
                                         hc[B][:, slc], tmp4[:, sl])
                    if defer_tail:
                        return tail
                    tail()

                # ------- build: d2 on PE, fused bracket counts, r0 ---------
                with tc.tile_pool(name="bld", bufs=1) as bld, \
                     tc.tile_pool(name="bscrd", bufs=1) as bscrd, \
                     tc.tile_pool(name="bscra", bufs=1) as bscra, \
                     tc.tile_pool(name="sqs", bufs=2) as sqsp, \
                     tc.tile_pool(name="bps", bufs=5, space="PSUM") as bps, \
                     tc.tile_pool(name="bps2", bufs=2, space="PSUM") as bps2:
                    aug_a = bld.tile([4, N], BF16, tag="aug_a", name="aug_a")
                    aug_b = bld.tile([4, N], BF16, tag="aug_b", name="aug_b")
                    ones1n = bld.tile([1, N], BF16, tag="ones1n", name="ones1n")
                    sqh1 = bld.tile([1, N], BF16, tag="sqh1", name="sqh1")
                    sql1 = bld.tile([1, N], BF16, tag="sql1", name="sql1")
                    nc.vector.memset(ones1n[:], 1.0)
                    nc.vector.tensor_copy(sqh1[0:1, :], sqrow[0:1, :])
                    nc.vector.tensor_sub(sql1[0:1, :], sqrow[0:1, :],
                                         sqh1[0:1, :])
                    nc.vector.tensor_copy(aug_a[0:1, :], sqh1[0:1, :])
                    nc.sync.dma_start(aug_a[1:2, :], sql1[0:1, :])
                    nc.sync.dma_start(aug_a[2:3, :], ones1n[0:1, :])
                    nc.sync.dma_start(aug_a[3:4, :], ones1n[0:1, :])
                    nc.vector.tensor_copy(aug_b[0:1, :], ones1n[0:1, :])
                    nc.sync.dma_start(aug_b[1:2, :], ones1n[0:1, :])
                    nc.sync.dma_start(aug_b[2:3, :], sqh1[0:1, :])
                    nc.sync.dma_start(aug_b[3:4, :], sql1[0:1, :])
                    scrD = bscrd.tile([128, N], BF16, tag="scrD", name="scrD")
                    scrA = bscra.tile([128, N], BF16, tag="scrA", name="scrA")
                    pend_tail = []
                    for g, (g0, g1) in enumerate([(0, 8), (8, NCH)]):
                        for c in range(g0, g1):
                            d2c = D2[:, c * N:(c + 1) * N]
                            cs = slice(c * 128, (c + 1) * 128)
                            for j in range(NJT):
                                js = slice(j * 512, (j + 1) * 512)
                                zz_ps = bps.tile([128, 512], F32, tag="zz",
                                                 name="zz")
                                nc.tensor.matmul(zz_ps[:], vh[:, cs], zh[:, js],
                                                 start=True, stop=False)
                                nc.tensor.matmul(zz_ps[:], vh[:, cs], zl[:, js],
                                                 start=False, stop=False)
                                nc.tensor.matmul(zz_ps[:], vl[:, cs], zh[:, js],
                                                 start=False, stop=False)
                                nc.tensor.matmul(zz_ps[:], aug_a[:, cs],
                                                 aug_b[:, js],
                                                 start=False, stop=True)
                                nc.scalar.activation(
                                    d2c[:, js], zz_ps[:], Act.Relu)
                            sqs = sqsp.tile([128, 512], F32, tag="sqs",
                                            name="sqs")
                            nc.scalar.activation(
                                sqs[:],
                                d2c.rearrange("p (a b) -> p a b", b=4)[:, :, 0:1],
                                Act.Sqrt, accum_out=musum[:, c:c + 1])
                            if c == 0:
                                emit_norms(bps2)
                            # fused bracket counts: DVE at hi, ACT Sign at lo
                            nc.vector.tensor_scalar(
                                scrD[:], d2c, hc[0][:, c:c + 1], scalar2=None,
                                op0=Alu.is_le, op1=Alu.add,
                                accum_out=hc[0][:, NCH + c:NCH + c + 1])
                            nc.scalar.activation(scrA[:], d2c, Act.Sign,
                                                 bias=neglo[:, c:c + 1],
                                                 accum_out=lacc[:, c:c + 1])
                        gs = slice(g0, g1)
                        gsc = slice(NCH + g0, NCH + g1)
                        nc.vector.tensor_copy(chieff[0][:, gs],
                                              hc[0][:, gsc])
                        nc.vector.tensor_scalar(
                            lc[0][:, gsc], lacc[:, gs], -0.5,
                            scalar2=float(N) * 0.5, op0=Alu.mult, op1=Alu.add)
                        # round 0 for this group (group A overlaps build of B)
                        if g == 0:
                            emit_round(0, g0, g1, 8, scrD, scrA)
                        else:
                            pend_tail.append(emit_round(0, g0, g1, 4,
                                                        scrD, scrA,
                                                        defer_tail=True))
                zpool.__exit__(None, None, None)

                # ================= rounds 1..N_ROUNDS-1 =================
                with tc.tile_pool(name="rscrd", bufs=1) as rscrd, \
                     tc.tile_pool(name="rscra", bufs=1) as rscra, \
                     tc.tile_pool(name="fwg", bufs=1) as fwg:
                    rD = rscrd.tile([128, N], BF16, tag="rD", name="rD")
                    rA = rscra.tile([128, N], BF16, tag="rA", name="rA")
                    # software-pipelined rounds: each DVE-group half runs
                    # while the ACT group's counts of the previous half are
                    # still in flight; the ACT-group conversion+updates (DVE
                    # ops) are deferred past the next DVE-group half so the
                    # in-order DVE queue never waits on ACT
                    for r in range(1, N_ROUNDS):
                        emit_round(r, 0, DVE_CNT, DVE_CNT, rD, rA)
                        pend_tail.pop(0)()
                        pend_tail.append(
                            emit_round(r, DVE_CNT, NCH, 0, rD, rA,
                                       defer_tail=True))

                    FI = N_ROUNDS % 2
                    hiF = hc[FI][:, 0:NCH]
                    chiF = hc[FI][:, NCH:32]
                    # ---- finisher: top-8 below hi; j = chi - K (tracked) ----
                    # ACT/GPS chunks first with dedicated scratch so the GPS
                    # pipeline starts immediately alongside DVE's STT chain.
                    scrF = fwg.tile([128, N], F32, tag="scrF", name="scrF")
                    scrF2 = fwg.tile([128, N], F32, tag="scrF2", name="scrF2")
                    nc.gpsimd.iota(iota8f[:], pattern=[[0, NCH], [1, 8]],
                                   base=0, channel_multiplier=0,
                                   allow_small_or_imprecise_dtypes=True)
                    if ACT_WSCR:
                        scrG = [fwg.tile([128, N], F32, tag=f"scrG{i}",
                                         name=f"scrG{i}") for i in range(3)]
                        indt = [fwg.tile([128, N], BF16, tag=f"indt{i}",
                                         name=f"indt{i}") for i in range(2)]
                        nc.vector.tensor_scalar(bighi[:], hiF, BIGSIG,
                                                scalar2=None, op0=Alu.mult)
                    def fin_chunk(c):
                        d2c = D2[:, c * N:(c + 1) * N]
                        scr = scrF if c % 2 == 0 else scrF2
                        nc.vector.scalar_tensor_tensor(
                            out=scr[:], in0=d2c, scalar=hiF[:, c:c + 1],
                            in1=d2c, op0=Alu.is_le, op1=Alu.mult)
                        nc.vector.max(out=w8[:, c * 8:(c + 1) * 8], in_=scr[:])
                    # DVE-group chunks use hi from round-3-A (already final);
                    # the ACT-group's deferred tail lands behind them, then
                    # the j-select prep (needs full chi) and the rest.
                    for c in range(DVE_CNT):
                        fin_chunk(c)
                    pend_tail.pop(0)()
                    nc.vector.tensor_scalar(tmp1[:], chiF, float(-K),
                                            scalar2=None, op0=Alu.add)
                    nc.vector.tensor_scalar(mge[:], tmp1[:], 0.0, scalar2=None,
                                            op0=Alu.is_ge)
                    nc.vector.tensor_scalar(mbh[:], tmp1[:], 7.0, scalar2=None,
                                            op0=Alu.is_le)
                    nc.vector.tensor_tensor(mok[:], mge[:], mbh[:],
                                            op=Alu.logical_and)
                    nc.vector.tensor_tensor(
                        ohsel[:].rearrange("p (c i) -> p c i", i=8),
                        iota8f[:].rearrange("p (c i) -> p c i", i=8),
                        tmp1[:].unsqueeze(2).to_broadcast([128, NCH, 8]),
                        op=Alu.is_equal)
                    for c in range(DVE_CNT, NCH):
                        fin_chunk(c)
                    nc.vector.tensor_mul(ohsel[:], ohsel[:], w8[:])
                    nc.vector.tensor_reduce(
                        tmp3[:], ohsel[:].rearrange("p (c i) -> p c i", i=8),
                        axis=X_AX, op=Alu.add)
                    # fallback: j>7 -> w8[7] (rank chi-7); j<0 -> hi
                    w87 = w8[:].rearrange("p (c i) -> p c i", i=8)[:, :, 7:8].squeeze(2)
                    nc.vector.select(tmp4[:], mge[:], w87, hiF)
                    nc.vector.select(Tfin[:], mok[:], tmp3[:], tmp4[:])

                # ---------------- mean distance ----------------
                with tc.tile_pool(name="mps", bufs=2, space="PSUM") as mps:
                    nc.vector.tensor_reduce(s_vec[:], musum[:], axis=X_AX,
                                            op=Alu.add)
                    ms_ps = mps.tile([1, 1], F32, tag="m", name="msps")
                    nc.tensor.matmul(ms_ps[:], ones_col[:], s_vec[:],
                                     start=True, stop=True)
                    nc.scalar.activation(sc2[:], ms_ps[:], Act.Copy,
                                         scale=1.0 / (N * 512.0))
                    nc.vector.tensor_reduce(
                        sc3[:],
                        sqrow[0:1, :].rearrange("p (a b) -> p a b", b=4)[:, :, 0:1],
                        axis=XY_AX, op=Alu.add)
                    nc.vector.tensor_scalar(sc3[:], sc3[:], 1.0 / 512.0,
                                            scalar2=None, op0=Alu.mult)
                    nc.vector.scalar_tensor_tensor(
                        out=sc4[:], in0=sc1[:], scalar=1.0 / N, in1=sc3[:],
                        op0=Alu.mult, op1=Alu.subtract)
                    nc.vector.tensor_scalar(sc3[:], sc2[:], 2.0, scalar2=None,
                                            op0=Alu.mult)
                    nc.vector.reciprocal(sc3[:], sc3[:])
                    nc.vector.tensor_mul(sc4[:], sc4[:], sc3[:])
                    nc.vector.tensor_add(sc2[:], sc2[:], sc4[:])
                    nc.vector.tensor_mul(sc2[:], sc2[:], sc2[:])
                    nc.vector.tensor_scalar(sc2[:], sc2[:], 2.0, scalar2=1e-8,
                                            op0=Alu.mult, op1=Alu.add)
                    nc.vector.reciprocal(sc2[:], sc2[:])
                    nc.vector.tensor_scalar(sc2[:], sc2[:], -1.0, scalar2=None,
                                            op0=Alu.mult)
                    ni_ps = mps.tile([128, 1], F32, tag="m", name="nips")
                    nc.tensor.matmul(ni_ps[:], ones_row[:], sc2[:],
                                     start=True, stop=True)
                    nc.vector.tensor_copy(neginvb[:], ni_ps[:])

            # ================= final phase =================
            with tc.tile_pool(name="fin1", bufs=1) as fin1, \
                 tc.tile_pool(name="fsim", bufs=3) as fsim, \
                 tc.tile_pool(name="fmsk", bufs=3) as fmsk, \
                 tc.tile_pool(name="fps", bufs=2, space="PSUM") as fps, \
                 tc.tile_pool(name="fps1", bufs=2, space="PSUM") as fps1:
                TROWB = fin1.tile([128, N], F32, tag="TROWB", name="TROWB")
                trow = fin1.tile([1, N], F32, tag="trow", name="trow")
                tcol = fin1.tile([16, 128], F32, tag="tcol", name="tcol")
                tf_ps = fps1.tile([16, 128], F32, tag="tfp", name="tfp")
                nc.tensor.transpose(tf_ps[:], Tfin[:], ident[:])
                nc.vector.tensor_copy(tcol[:], tf_ps[:])
                dqs = [nc.sync, nc.gpsimd, nc.scalar]
                for c in range(NCH):
                    dqs[c % 3].dma_start(trow[0:1, c * 128:(c + 1) * 128],
                                         tcol[c:c + 1, :])
                for j in range(NJT):
                    tb_ps = fps.tile([128, 512], F32, tag="tbps", name="tbps")
                    nc.tensor.matmul(tb_ps[:], ones_row[:],
                                     trow[0:1, j * 512:(j + 1) * 512],
                                     start=True, stop=True)
                    nc.vector.tensor_copy(TROWB[:, j * 512:(j + 1) * 512],
                                          tb_ps[:])
                for c in range(NCH):
                    d2c = D2[:, c * N:(c + 1) * N]
                    simt = fsim.tile([128, N], BF16, tag="simt", name="simt")
                    nc.scalar.activation(simt[:], d2c, Act.Exp, bias=0.0,
                                         scale=neginvb[:])
                    ms = fmsk.tile([128, N], BF16, tag="ms", name="ms")
                    nc.vector.scalar_tensor_tensor(
                        out=ms[:], in0=TROWB[:], scalar=Tfin[:, c:c + 1],
                        in1=d2c, op0=Alu.max, op1=Alu.is_ge)
                    nc.vector.tensor_mul(simt[:], ms[:], simt[:])
                    nc.sync.dma_start(outp[c * 128:(c + 1) * 128, :], simt[:])
    nc.compile()
    return nc


_NC_CACHE = None
LAST_RESULTS = None
_IDM = np.ascontiguousarray(np.eye(128, dtype=np.float32))


def _get_nc():
    global _NC_CACHE
    if _NC_CACHE is None:
        _NC_CACHE = build_nc()
    return _NC_CACHE


def kernel(x, W):
    from concourse.bass_utils import run_bass_kernel_spmd
    x = np.ascontiguousarray(np.asarray(x, dtype=np.float32))
    W = np.ascontiguousarray(np.asarray(W, dtype=np.float32))
    nc = _get_nc()
    in_maps = []
    for i in range(8):
        h, b = i // 2, i % 2
        in_maps.append({"xb": np.ascontiguousarray(x[b]),
                        "wh": np.ascontiguousarray(W[h]),
                        "idm": _IDM})
    res = run_bass_kernel_spmd(nc, in_maps, core_ids=list(range(8)))
    global LAST_RESULTS
    LAST_RESULTS = res
    C = [np.asarray(res.results[i]["outp"]).astype(np.float32)
         for i in range(8)]
    adj = np.stack([
        (C[0 + b] + C[2 + b] + C[4 + b] + C[6 + b]) * 0.25 for b in range(2)
    ]).astype(np.float32)
    return adj


# revision 53
# speedup vs baseline: 1.0671x; 1.0671x over previous
"""MultiHeadDistanceKNN Trainium2 kernel.

kernel(x, W) -> adj : x [2,2048,512] f32, W [4,512,128] f32 -> adj [2,2048,2048] f32.

8 cores = 4 heads x 2 batches; core i handles (h=i//2, b=i%2) and computes
  C[n,m] = exp(-d2[n,m]/(2*mu^2)) * 1[d2[n,m] <= max(T_n, T_m)]
where d2 = |z_n - z_m|^2 for z = x_b @ W_h (exactly symmetric), T_n = K-th
smallest of row n (K=307), mu = mean distance. Host: adj[b] = mean_h C.

Pipeline: prep streams x chunks (sync DMA queue), transposes them on the PE
and fires each z j-tile as soon as its 4 chunks land. d2 built on the PE
(bf16-split 3-term matmul + K=4 aug for the norms), ACT relu-copies psum ->
SBUF D2. Bracket counts are fused into the build (DVE is_le+accum at hi, ACT
Sign+accum at lo -> exact clo); the build runs in two 8-chunk groups so probe
round 0 for group A hides under group B's matmuls. 4 more secant rounds
(DVE/ACT split with per-engine scratch), then a top-8-below-hi finisher:
wscr = (d2<=hi)*d2 in one DVE STT, MAX8, T = w8[chi - K] (no recount - chi
is tracked exactly through the rounds).
Final: sim = ACT exp (bf16), mask = STT(max(TROWB,T_n) >= d2) bf16, C = bf16
tensor_mul (DVE 2x mode; GpSimd deliberately idle - its big ops stall DVE via
the shared SBUF ports), bf16 DMA out; host upcasts and head-means. The
identity for PE transposes ships as a kernel input (GPS setup ops would
delay the first transposes ~8us).
"""
import numpy as np

import concourse.bass as bass
import concourse.mybir as mybir
from concourse import bacc
from concourse.tile import TileContext

F32 = mybir.dt.float32
BF16 = mybir.dt.bfloat16
U8 = mybir.dt.uint8
Alu = mybir.AluOpType
Act = mybir.ActivationFunctionType
X_AX = mybir.AxisListType.X
XY_AX = mybir.AxisListType.XY

N = 2048
D = 512
NCH = 16
NJT = 4
K = 307

N_ROUNDS = 4          # probe rounds total (round 0 split across build groups)
AIM = 3.5
ZLO = -1.45
ZHI = -0.70
DVE_CNT = 7           # count chunks on DVE per full round (rest on ACT)
ACT_WSCR = 0          # finisher wscr chunks via ACT sigmoid + GPS mult
BIGSIG = 4.0e4


def build_nc():
    nc = bacc.Bacc("TRN2", target_bir_lowering=False)
    xb = nc.dram_tensor("xb", [N, D], F32, kind="ExternalInput")
    wh = nc.dram_tensor("wh", [D, 128], F32, kind="ExternalInput")
    idm = nc.dram_tensor("idm", [128, 128], F32, kind="ExternalInput")
    outp = nc.dram_tensor("outp", [N, N], BF16, kind="ExternalOutput")

    with TileContext(nc) as tc:
        with tc.tile_pool(name="base", bufs=1) as base, \
             tc.tile_pool(name="st", bufs=1) as st:
            D2 = base.tile([128, NCH * N], F32)
            ident = base.tile([128, 128], F32)
            nc.sync.dma_start(ident[:], idm[:, :])
            ones_col = base.tile([128, 1], F32)
            nc.vector.memset(ones_col[:], 1.0)
            ones_row = base.tile([1, 128], F32)
            nc.vector.memset(ones_row[:], 1.0)
            id1 = base.tile([1, 1], F32)
            nc.vector.memset(id1[:], 1.0)
            c256 = base.tile([128, 1], F32)
            nc.vector.memset(c256[:], 256.0)

            def stt16(name, w=NCH, dt=F32):
                return st.tile([128, w], dt, tag=name, name=name)
            sqcol = stt16("sqcol"); zdots = stt16("zdots")
            mu = stt16("mu"); sig = stt16("sig")
            width0 = stt16("width0")
            # paired state, ping-pong: [:, 0:16] = value, [:, 16:32] = count
            lc = [stt16("lcA", 32), stt16("lcB", 32)]     # (lo, clo)
            hc = [stt16("hcA", 32), stt16("hcB", 32)]     # (hi, chi)
            chieff = [stt16("ceA"), stt16("ceB")]
            tcnt = stt16("tcnt", 32)                       # (t, cnt)
            negt = stt16("negt"); sacc = stt16("sacc")
            neglo = stt16("neglo"); lacc = stt16("lacc")
            musum = stt16("musum")
            Tfin = stt16("Tfin")
            bighi = stt16("bighi")
            tmp1 = stt16("tmp1"); tmp2 = stt16("tmp2"); tmp3 = stt16("tmp3")
            tmp4 = stt16("tmp4")
            mge = stt16("mge", NCH, U8)
            mbh = stt16("mbh", NCH, U8)
            mok = stt16("mok", NCH, U8)
            s_vec = st.tile([128, 1], F32, tag="s_vec", name="s_vec")
            s1b = st.tile([128, 1], F32, tag="s1b", name="s1b")
            neginvb = st.tile([128, 1], F32, tag="neginvb", name="neginvb")
            sc1 = st.tile([1, 1], F32, tag="sc1", name="sc1")
            sc2 = st.tile([1, 1], F32, tag="sc2", name="sc2")
            sc3 = st.tile([1, 1], F32, tag="sc3", name="sc3")
            sc4 = st.tile([1, 1], F32, tag="sc4", name="sc4")
            w8 = st.tile([128, NCH * 8], F32, tag="w8", name="w8")
            iota8f = st.tile([128, NCH * 8], F32, tag="iota8f", name="iota8f")
            ohsel = st.tile([128, NCH * 8], F32, tag="ohsel", name="ohsel")

            # ============== prep: stream x, transpose, z, norms ==============
            with tc.tile_pool(name="mid", bufs=1) as mid:
                sqrow = mid.tile([1, N], F32, tag="sqrow", name="sqrow")
                zpool = tc.tile_pool(name="zp", bufs=1)
                zp = zpool.__enter__()
                zT = zp.tile([128, N], F32, tag="zT", name="zT")
                zh = zp.tile([128, N], BF16, tag="zh", name="zh")
                zl = zp.tile([128, N], BF16, tag="zl", name="zl")
                vh = zp.tile([128, N], BF16, tag="vh", name="vh")
                vl = zp.tile([128, N], BF16, tag="vl", name="vl")

                with tc.tile_pool(name="prep", bufs=3) as prep, \
                     tc.tile_pool(name="prep1", bufs=1) as prep1, \
                     tc.tile_pool(name="pps", bufs=2, space="PSUM") as pps, \
                     tc.tile_pool(name="pps1", bufs=4, space="PSUM") as pps1:
                    w_sb = prep1.tile([128, D], F32)
                    xt = [prep1.tile([128, N], F32, tag=f"xt{dc}", name=f"xt{dc}")
                          for dc in range(4)]
                    for j in range(NJT):
                        js = slice(j * 512, (j + 1) * 512)
                        for cc in range(4):
                            c = 4 * j + cc
                            x_sb = prep.tile([128, D], F32, tag="x_sb",
                                             name="x_sb")
                            nc.sync.dma_start(x_sb[:],
                                              xb[c * 128:(c + 1) * 128, :])
                            for dc in range(4):
                                tr_ps = pps1.tile([128, 128], F32, tag="small",
                                                  name="tr")
                                nc.tensor.transpose(
                                    tr_ps[:], x_sb[:, dc * 128:(dc + 1) * 128],
                                    ident[:])
                                if dc % 2 == 0:
                                    nc.scalar.copy(
                                        xt[dc][:, c * 128:(c + 1) * 128],
                                        tr_ps[:])
                                else:
                                    nc.vector.tensor_copy(
                                        xt[dc][:, c * 128:(c + 1) * 128],
                                        tr_ps[:])
                        if j == 0:
                            for dc in range(4):
                                nc.sync.dma_start(
                                    w_sb[:, dc * 128:(dc + 1) * 128],
                                    wh[dc * 128:(dc + 1) * 128, :])
                        zt_ps = pps.tile([128, 512], F32, tag="big", name="zt")
                        for dc in range(4):
                            nc.tensor.matmul(
                                zt_ps[:], w_sb[:, dc * 128:(dc + 1) * 128],
                                xt[dc][:, js],
                                start=(dc == 0), stop=(dc == 3))
                        nc.vector.tensor_copy(zT[:, js], zt_ps[:])
                        # bf16 split per j-tile (DVE, overlaps PE)
                        nc.vector.tensor_copy(zh[:, js], zT[:, js])
                        nc.vector.tensor_sub(zl[:, js], zT[:, js], zh[:, js])
                        nc.vector.tensor_scalar(vh[:, js], zh[:, js], -2.0,
                                                scalar2=None, op0=Alu.mult)
                        nc.vector.tensor_scalar(vl[:, js], zl[:, js], -2.0,
                                                scalar2=None, op0=Alu.mult)
                        # squared norms per j-tile (ACT square into D2 scratch)
                        zT2j = D2[:, j * 512:(j + 1) * 512]
                        nc.scalar.activation(zT2j, zT[:, js], Act.Square)
                        sq_ps = pps1.tile([1, 512], F32, tag="small",
                                          name="sqps")
                        nc.tensor.matmul(sq_ps[:], ones_col[:], zT2j,
                                         start=True, stop=True)
                        nc.vector.tensor_copy(sqrow[0:1, js], sq_ps[:])

                # ---- norms + bracket init (emitted inside the build so
                # its serial math hides under chunk-0's matmuls) ----
                def emit_norms(pp):
                    for c in range(NCH):
                        tp = pp.tile([128, 1], F32, tag="small", name="sqcolp")
                        nc.tensor.transpose(tp[:],
                                            sqrow[0:1, c * 128:(c + 1) * 128],
                                            id1[:])
                        nc.vector.tensor_copy(sqcol[:, c:c + 1], tp[:])
                    nc.vector.tensor_reduce(s_vec[:], zT[:], axis=X_AX,
                                            op=Alu.add)
                    for c in range(NCH):
                        zd_ps = pp.tile([128, 1], F32, tag="small", name="zdps")
                        nc.tensor.matmul(zd_ps[:], zT[:, c * 128:(c + 1) * 128],
                                         s_vec[:], start=True, stop=True)
                        nc.vector.tensor_copy(zdots[:, c:c + 1], zd_ps[:])
                    nc.vector.tensor_reduce(sc1[:], sqrow[0:1, :], axis=X_AX,
                                            op=Alu.add)
                    s1_ps = pp.tile([128, 1], F32, tag="small", name="s1ps")
                    nc.tensor.matmul(s1_ps[:], ones_row[:], sc1[:],
                                     start=True, stop=True)
                    nc.vector.tensor_scalar(s1b[:], s1_ps[:], 1.0 / N,
                                            scalar2=None, op0=Alu.mult)
                    nc.vector.scalar_tensor_tensor(
                        out=mu[:], in0=zdots[:], scalar=-2.0 / N, in1=sqcol[:],
                        op0=Alu.mult, op1=Alu.add)
                    nc.vector.tensor_scalar(mu[:], mu[:], s1b[:], scalar2=None,
                                            op0=Alu.add)
                    # sig = sqrt(4*sqcol+256) linearized (keeps the norm
                    # chain DVE-only so ACT's in-order queue never head-blocks)
                    nc.vector.tensor_scalar(sig[:], sqcol[:], 0.07239054,
                                            scalar2=18.446953,
                                            op0=Alu.mult, op1=Alu.add)
                    nc.vector.scalar_tensor_tensor(
                        out=lc[0][:, 0:NCH], in0=sig[:], scalar=ZLO, in1=mu[:],
                        op0=Alu.mult, op1=Alu.add)
                    nc.vector.scalar_tensor_tensor(
                        out=hc[0][:, 0:NCH], in0=sig[:], scalar=ZHI, in1=mu[:],
                        op0=Alu.mult, op1=Alu.add)
                    nc.vector.tensor_sub(width0[:], hc[0][:, 0:NCH],
                                         lc[0][:, 0:NCH])
                    nc.vector.tensor_scalar(neglo[:], lc[0][:, 0:NCH], -1.0,
                                            scalar2=None, op0=Alu.mult)

                # ---- secant round helper (column-sliced) ----
                def emit_round(r, c0, c1, dve_k, cntD, cntA_scr):
                    A, B = r % 2, (r + 1) % 2
                    sl = slice(c0, c1)
                    slc = slice(NCH + c0, NCH + c1)
                    loA, cloA = lc[A][:, sl], lc[A][:, slc]
                    hiA, chiA = hc[A][:, sl], hc[A][:, slc]
                    tA, cntsl = tcnt[:, sl], tcnt[:, slc]
                    # t = lo + clamp((K+AIM-clo)/(chieff-clo), .02, .98)*(hi-lo)
                    nc.vector.tensor_sub(tmp1[:, sl], chieff[A][:, sl], cloA)
                    nc.vector.tensor_scalar(tmp1[:, sl], tmp1[:, sl], 1.0,
                                            scalar2=None, op0=Alu.max)
                    nc.vector.reciprocal(tmp1[:, sl], tmp1[:, sl])
                    nc.vector.tensor_scalar(tmp2[:, sl], cloA, -1.0,
                                            scalar2=float(K) + AIM,
                                            op0=Alu.mult, op1=Alu.add)
                    nc.vector.tensor_mul(tmp1[:, sl], tmp1[:, sl], tmp2[:, sl])
                    nc.vector.tensor_scalar(tmp1[:, sl], tmp1[:, sl], 0.02,
                                            scalar2=0.98,
                                            op0=Alu.max, op1=Alu.min)
                    nc.vector.tensor_sub(tmp2[:, sl], hiA, loA)
                    nc.vector.tensor_mul(tmp1[:, sl], tmp1[:, sl], tmp2[:, sl])
                    nc.vector.tensor_add(tA, tmp1[:, sl], loA)
                    repair = r < 4
                    if repair:
                        nc.vector.tensor_scalar(mbh[:, sl], chiA, float(K),
                                                scalar2=None, op0=Alu.is_lt)
                        nc.vector.scalar_tensor_tensor(
                            out=tmp3[:, sl], in0=width0[:, sl],
                            scalar=float(2.0 ** r),
                            in1=hiA, op0=Alu.mult, op1=Alu.add)
                        nc.vector.select(tA, mbh[:, sl], tmp3[:, sl], tA)
                    nc.vector.tensor_scalar(negt[:, sl], tA, -1.0,
                                            scalar2=None, op0=Alu.mult)
                    # counts: first dve_k chunks on DVE, rest on ACT
                    act_cs = []
                    for c in range(c0, c1):
                        d2c = D2[:, c * N:(c + 1) * N]
                        if c - c0 < dve_k:
                            nc.vector.tensor_scalar(
                                cntD[:], d2c, tcnt[:, c:c + 1], scalar2=None,
                                op0=Alu.is_le, op1=Alu.add,
                                accum_out=tcnt[:, NCH + c:NCH + c + 1])
                        else:
                            nc.scalar.activation(cntA_scr[:], d2c, Act.Sign,
                                                 bias=negt[:, c:c + 1],
                                                 accum_out=sacc[:, c:c + 1])
                            act_cs.append(c)
                    if act_cs:
                        a0, a1 = act_cs[0], act_cs[-1] + 1
                        nc.vector.tensor_scalar(
                            tcnt[:, NCH + a0:NCH + a1], sacc[:, a0:a1], -0.5,
                            scalar2=float(N) * 0.5, op0=Alu.mult, op1=Alu.add)
                    # updates -> B set
                    nc.vector.tensor_scalar(mge[:, sl], cntsl, float(K),
                                            scalar2=None, op0=Alu.is_ge)
                    nc.vector.select(lc[B][:, sl], mge[:, sl], loA, tA)
                    nc.vector.select(lc[B][:, slc], mge[:, sl], cloA, cntsl)
                    nc.vector.select(hc[B][:, sl], mge[:, sl], tA, hiA)
                    nc.vector.select(hc[B][:, slc], mge[:, sl], cntsl, chiA)
                    if repair:
                        nc.vector.select(lc[B][:, sl], mbh[:, sl], hiA,
                                         lc[B][:, sl])
                        nc.vector.select(lc[B][:, slc], mbh[:, sl], chiA,
                                         lc[B][:, slc])
                        nc.vector.select(hc[B][:, sl], mbh[:, sl], tA,
                                         hc[B][:, sl])
                        nc.vector.select(hc[B][:, slc], mbh[:, sl], cntsl,
                                         hc[B][:, slc])
                    nc.vector.tensor_scalar(tmp4[:, sl], chieff[A][:, sl], 0.5,
                                            scalar2=float(K) * 0.5,
                                            op0=Alu.mult, op1=Alu.add)
                    nc.vector.select(chieff[B][:, sl], mge[:, sl],
                                     hc[B][:, slc], tmp4[:, sl])

                # ------- build: d2 on PE, fused bracket counts, r0 ---------
                with tc.tile_pool(name="bld", bufs=1) as bld, \
                     tc.tile_pool(name="bscrd", bufs=1) as bscrd, \
                     tc.tile_pool(name="bscra", bufs=1) as bscra, \
                     tc.tile_pool(name="sqs", bufs=2) as sqsp, \
                     tc.tile_pool(name="bps", bufs=5, space="PSUM") as bps, \
                     tc.tile_pool(name="bps2", bufs=2, space="PSUM") as bps2:
                    aug_a = bld.tile([4, N], BF16, tag="aug_a", name="aug_a")
                    aug_b = bld.tile([4, N], BF16, tag="aug_b", name="aug_b")
                    ones1n = bld.tile([1, N], BF16, tag="ones1n", name="ones1n")
                    sqh1 = bld.tile([1, N], BF16, tag="sqh1", name="sqh1")
                    sql1 = bld.tile([1, N], BF16, tag="sql1", name="sql1")
                    nc.vector.memset(ones1n[:], 1.0)
                    nc.vector.tensor_copy(sqh1[0:1, :], sqrow[0:1, :])
                    nc.vector.tensor_sub(sql1[0:1, :], sqrow[0:1, :],
                                         sqh1[0:1, :])
                    nc.vector.tensor_copy(aug_a[0:1, :], sqh1[0:1, :])
                    nc.sync.dma_start(aug_a[1:2, :], sql1[0:1, :])
                    nc.sync.dma_start(aug_a[2:3, :], ones1n[0:1, :])
                    nc.sync.dma_start(aug_a[3:4, :], ones1n[0:1, :])
                    nc.vector.tensor_copy(aug_b[0:1, :], ones1n[0:1, :])
                    nc.sync.dma_start(aug_b[1:2, :], ones1n[0:1, :])
                    nc.sync.dma_start(aug_b[2:3, :], sqh1[0:1, :])
                    nc.sync.dma_start(aug_b[3:4, :], sql1[0:1, :])
                    scrD = bscrd.tile([128, N], BF16, tag="scrD", name="scrD")
                    scrA = bscra.tile([128, N], BF16, tag="scrA", name="scrA")
                    for g, (g0, g1) in enumerate([(0, 8), (8, NCH)]):
                        for c in range(g0, g1):
                            d2c = D2[:, c * N:(c + 1) * N]
                            cs = slice(c * 128, (c + 1) * 128)
                            for j in range(NJT):
                                js = slice(j * 512, (j + 1) * 512)
                                zz_ps = bps.tile([128, 512], F32, tag="zz",
                                                 name="zz")
                                nc.tensor.matmul(zz_ps[:], vh[:, cs], zh[:, js],
                                                 start=True, stop=False)
                                nc.tensor.matmul(zz_ps[:], vh[:, cs], zl[:, js],
                                                 start=False, stop=False)
                                nc.tensor.matmul(zz_ps[:], vl[:, cs], zh[:, js],
                                                 start=False, stop=False)
                                nc.tensor.matmul(zz_ps[:], aug_a[:, cs],
                                                 aug_b[:, js],
                                                 start=False, stop=True)
                                nc.scalar.activation(
                                    d2c[:, js], zz_ps[:], Act.Relu)
                            sqs = sqsp.tile([128, 512], F32, tag="sqs",
                                            name="sqs")
                            nc.scalar.activation(
                                sqs[:],
                                d2c.rearrange("p (a b) -> p a b", b=4)[:, :, 0:1],
                                Act.Sqrt, accum_out=musum[:, c:c + 1])
                            if c == 0:
                                emit_norms(bps2)
                            # fused bracket counts: DVE at hi, ACT Sign at lo
                            nc.vector.tensor_scalar(
                                scrD[:], d2c, hc[0][:, c:c + 1], scalar2=None,
                                op0=Alu.is_le, op1=Alu.add,
                                accum_out=hc[0][:, NCH + c:NCH + c + 1])
                            nc.scalar.activation(scrA[:], d2c, Act.Sign,
                                                 bias=neglo[:, c:c + 1],
                                                 accum_out=lacc[:, c:c + 1])
                        gs = slice(g0, g1)
                        gsc = slice(NCH + g0, NCH + g1)
                        nc.vector.tensor_copy(chieff[0][:, gs],
                                              hc[0][:, gsc])
                        nc.vector.tensor_scalar(
                            lc[0][:, gsc], lacc[:, gs], -0.5,
                            scalar2=float(N) * 0.5, op0=Alu.mult, op1=Alu.add)
                        # round 0 for this group (group A overlaps build of B)
                        emit_round(0, g0, g1, 8 if g == 0 else 4,
                                   scrD, scrA)
                zpool.__exit__(None, None, None)

                # ================= rounds 1..N_ROUNDS-1 =================
                with tc.tile_pool(name="rscrd", bufs=1) as rscrd, \
                     tc.tile_pool(name="rscra", bufs=1) as rscra, \
                     tc.tile_pool(name="fwg", bufs=1) as fwg:
                    rD = rscrd.tile([128, N], BF16, tag="rD", name="rD")
                    rA = rscra.tile([128, N], BF16, tag="rA", name="rA")
                    for r in range(1, N_ROUNDS):
                        emit_round(r, 0, NCH, DVE_CNT, rD, rA)

                    FI = N_ROUNDS % 2
                    hiF = hc[FI][:, 0:NCH]
                    chiF = hc[FI][:, NCH:32]
                    # ---- finisher: top-8 below hi; j = chi - K (tracked) ----
                    # ACT/GPS chunks first with dedicated scratch so the GPS
                    # pipeline starts immediately alongside DVE's STT chain.
                    scrF = fwg.tile([128, N], F32, tag="scrF", name="scrF")
                    scrF2 = fwg.tile([128, N], F32, tag="scrF2", name="scrF2")
                    nc.gpsimd.iota(iota8f[:], pattern=[[0, NCH], [1, 8]],
                                   base=0, channel_multiplier=0,
                                   allow_small_or_imprecise_dtypes=True)
                    if ACT_WSCR:
                        scrG = [fwg.tile([128, N], F32, tag=f"scrG{i}",
                                         name=f"scrG{i}") for i in range(3)]
                        indt = [fwg.tile([128, N], BF16, tag=f"indt{i}",
                                         name=f"indt{i}") for i in range(2)]
                        nc.vector.tensor_scalar(bighi[:], hiF, BIGSIG,
                                                scalar2=None, op0=Alu.mult)
                    # j-select prep (depends only on chi from the rounds,
                    # not on w8 - emit ahead of the DVE-saturated wscr chain)
                    nc.vector.tensor_scalar(tmp1[:], chiF, float(-K),
                                            scalar2=None, op0=Alu.add)
                    nc.vector.tensor_scalar(mge[:], tmp1[:], 0.0, scalar2=None,
                                            op0=Alu.is_ge)
                    nc.vector.tensor_scalar(mbh[:], tmp1[:], 7.0, scalar2=None,
                                            op0=Alu.is_le)
                    nc.vector.tensor_tensor(mok[:], mge[:], mbh[:],
                                            op=Alu.logical_and)
                    nc.vector.tensor_tensor(
                        ohsel[:].rearrange("p (c i) -> p c i", i=8),
                        iota8f[:].rearrange("p (c i) -> p c i", i=8),
                        tmp1[:].unsqueeze(2).to_broadcast([128, NCH, 8]),
                        op=Alu.is_equal)
                    # GPS path: ACT sigmoid ind -> GPS mult, 3-deep scratch
                    # rotation; their MAX8s interleave into the DVE chain with
                    # a 2-pair lag so neither engine stalls the other.
                    for c in range(ACT_WSCR):
                        d2c = D2[:, c * N:(c + 1) * N]
                        ind = indt[c % 2]
                        nc.scalar.activation(ind[:], d2c, Act.Sigmoid,
                                             bias=bighi[:, c:c + 1],
                                             scale=-BIGSIG)
                        nc.gpsimd.tensor_mul(scrG[c % 3][:], ind[:], d2c)
                    pend = list(range(ACT_WSCR))
                    for i, c in enumerate(range(ACT_WSCR, NCH)):
                        d2c = D2[:, c * N:(c + 1) * N]
                        scr = scrF if c % 2 == 0 else scrF2
                        nc.vector.scalar_tensor_tensor(
                            out=scr[:], in0=d2c, scalar=hiF[:, c:c + 1],
                            in1=d2c, op0=Alu.is_le, op1=Alu.mult)
                        nc.vector.max(out=w8[:, c * 8:(c + 1) * 8], in_=scr[:])
                        if i >= 1 and pend:
                            g = pend.pop(0)
                            nc.vector.max(out=w8[:, g * 8:(g + 1) * 8],
                                          in_=scrG[g % 3][:])
                    for g in pend:
                        nc.vector.max(out=w8[:, g * 8:(g + 1) * 8],
                                      in_=scrG[g % 3][:])
                    nc.vector.tensor_mul(ohsel[:], ohsel[:], w8[:])
                    nc.vector.tensor_reduce(
                        tmp3[:], ohsel[:].rearrange("p (c i) -> p c i", i=8),
                        axis=X_AX, op=Alu.add)
                    # fallback: j>7 -> w8[7] (rank chi-7); j<0 -> hi
                    w87 = w8[:].rearrange("p (c i) -> p c i", i=8)[:, :, 7:8].squeeze(2)
                    nc.vector.select(tmp4[:], mge[:], w87, hiF)
                    nc.vector.select(Tfin[:], mok[:], tmp3[:], tmp4[:])

                # ---------------- mean distance ----------------
                with tc.tile_pool(name="mps", bufs=2, space="PSUM") as mps:
                    nc.vector.tensor_reduce(s_vec[:], musum[:], axis=X_AX,
                                            op=Alu.add)
                    ms_ps = mps.tile([1, 1], F32, tag="m", name="msps")
                    nc.tensor.matmul(ms_ps[:], ones_col[:], s_vec[:],
                                     start=True, stop=True)
                    nc.scalar.activation(sc2[:], ms_ps[:], Act.Copy,
                                         scale=1.0 / (N * 512.0))
                    nc.vector.tensor_reduce(
                        sc3[:],
                        sqrow[0:1, :].rearrange("p (a b) -> p a b", b=4)[:, :, 0:1],
                        axis=XY_AX, op=Alu.add)
                    nc.vector.tensor_scalar(sc3[:], sc3[:], 1.0 / 512.0,
                                            scalar2=None, op0=Alu.mult)
                    nc.vector.scalar_tensor_tensor(
                        out=sc4[:], in0=sc1[:], scalar=1.0 / N, in1=sc3[:],
                        op0=Alu.mult, op1=Alu.subtract)
                    nc.vector.tensor_scalar(sc3[:], sc2[:], 2.0, scalar2=None,
                                            op0=Alu.mult)
                    nc.vector.reciprocal(sc3[:], sc3[:])
                    nc.vector.tensor_mul(sc4[:], sc4[:], sc3[:])
                    nc.vector.tensor_add(sc2[:], sc2[:], sc4[:])
                    nc.vector.tensor_mul(sc2[:], sc2[:], sc2[:])
                    nc.vector.tensor_scalar(sc2[:], sc2[:], 2.0, scalar2=1e-8,
                                            op0=Alu.mult, op1=Alu.add)
                    nc.vector.reciprocal(sc2[:], sc2[:])
                    nc.vector.tensor_scalar(sc2[:], sc2[:], -1.0, scalar2=None,
                                            op0=Alu.mult)
                    ni_ps = mps.tile([128, 1], F32, tag="m", name="nips")
                    nc.tensor.matmul(ni_ps[:], ones_row[:], sc2[:],
                                     start=True, stop=True)
                    nc.vector.tensor_copy(neginvb[:], ni_ps[:])

            # ================= final phase =================
            with tc.tile_pool(name="fin1", bufs=1) as fin1, \
                 tc.tile_pool(name="fsim", bufs=3) as fsim, \
                 tc.tile_pool(name="fmsk", bufs=3) as fmsk, \
                 tc.tile_pool(name="fps", bufs=2, space="PSUM") as fps, \
                 tc.tile_pool(name="fps1", bufs=2, space="PSUM") as fps1:
                TROWB = fin1.tile([128, N], F32, tag="TROWB", name="TROWB")
                trow = fin1.tile([1, N], F32, tag="trow", name="trow")
                tcol = fin1.tile([16, 128], F32, tag="tcol", name="tcol")
                tf_ps = fps1.tile([16, 128], F32, tag="tfp", name="tfp")
                nc.tensor.transpose(tf_ps[:], Tfin[:], ident[:])
                nc.vector.tensor_copy(tcol[:], tf_ps[:])
                dqs = [nc.sync, nc.gpsimd, nc.scalar]
                for c in range(NCH):
                    dqs[c % 3].dma_start(trow[0:1, c * 128:(c + 1) * 128],
                                         tcol[c:c + 1, :])
                for j in range(NJT):
                    tb_ps = fps.tile([128, 512], F32, tag="tbps", name="tbps")
                    nc.tensor.matmul(tb_ps[:], ones_row[:],
                                     trow[0:1, j * 512:(j + 1) * 512],
                                     start=True, stop=True)
                    nc.vector.tensor_copy(TROWB[:, j * 512:(j + 1) * 512],
                                          tb_ps[:])
                for c in range(NCH):
                    d2c = D2[:, c * N:(c + 1) * N]
                    simt = fsim.tile([128, N], BF16, tag="simt", name="simt")
                    nc.scalar.activation(simt[:], d2c, Act.Exp, bias=0.0,
                                         scale=neginvb[:])
                    ms = fmsk.tile([128, N], BF16, tag="ms", name="ms")
                    nc.vector.scalar_tensor_tensor(
                        out=ms[:], in0=TROWB[:], scalar=Tfin[:, c:c + 1],
                        in1=d2c, op0=Alu.max, op1=Alu.is_ge)
                    nc.vector.tensor_mul(simt[:], ms[:], simt[:])
                    nc.sync.dma_start(outp[c * 128:(c + 1) * 128, :], simt[:])
    nc.compile()
    return nc


_NC_CACHE = None
LAST_RESULTS = None
_IDM = np.ascontiguousarray(np.eye(128, dtype=np.float32))


def _get_nc():
    global _NC_CACHE
    if _NC_CACHE is None:
        _NC_CACHE = build_nc()
    return _NC_CACHE


def kernel(x, W):
    from concourse.bass_utils import run_bass_kernel_spmd
    x = np.ascontiguousarray(np.asarray(x, dtype=np.float32))
    W = np.ascontiguousarray(np.asarray(W, dtype=np.float32))
    nc = _get_nc()
    in_maps = []
    for i in range(8):
        h, b = i // 2, i % 2
        in_maps.append({"xb": np.ascontiguousarray(x[b]),
                        "wh": np.ascontiguousarray(W[h]),
                        "idm": _IDM})
    res = run_bass_kernel_spmd(nc, in_maps, core_ids=list(range(8)))
    global LAST_RESULTS
    LAST_RESULTS = res
    C = [np.asarray(res.results[i]["outp"]).astype(np.float32)
         for i in range(8)]
    adj = np.stack([
        (C[0 + b] + C[2 + b] + C[4 + b] + C[6 + b]) * 0.25 for b in range(2)
    ]).astype(np.float32)
    return adj
